# revision 4
# baseline (speedup 1.0000x reference)
"""Trainium2 kernel for nn_BLInputLayer (SparseConvNet mode-3 input layer).

reference semantics: linearize each point's (batch, x, y, z) into a key,
jnp.unique the keys (sorted, size=n, fill -1), segment-sum features by the
inverse index.  Output row u is the feature-sum of the points at the u-th
smallest unique site key; rows past the number of unique sites are zero.

Distribution: data-parallel over the batch dim (8 batches -> 8 NeuronCores).
Keys are batch-major, so the globally sorted unique sites are the per-batch
sorted unique sites concatenated; the host packs the per-core results at the
right row offsets.

Device kernel (per core, raw Bass): the 32768 output slots are produced by
tiled SWDGE `dma_gather`s (one 512B feature row per slot, fetched from the
slot's first occurrence point) pipelined with contiguous HWDGE writes.  The
gathers are spread over 4 SWDGE queues so all 8 GPSIMD cores (one tx/rx pair
per queue) generate DMA descriptors concurrently -- descriptor generation on
a single pair is the sequential bottleneck (~8 ns/row).  The handful of
duplicate points per batch are pre-summed on the host into the uploaded copy
of that batch's features (first-occurrence rows), so the gather alone yields
the exact segment-sum and no device-side read-modify-write pass is needed.
Host work stays O(L) integer planning on coords plus O(dups * C) feature
touches; all bulk feature traffic (16.7 MB in + 16.7 MB out per core) is
on-device.
"""

import numpy as np

B, L, DIM, C = 8, 32768, 3, 128
S = 512
P = 128
NQ = 4                      # SWDGE queues == GPSIMD core pairs used
QTOK = L // NQ              # tokens (output slots) per queue
# Per-queue chunk schedule. A chunk's gather DMA only fires once its
# descriptor generation ends, so tiny front chunks get the DMA engines going
# early; big middle chunks amortize per-instruction overhead; the taper keeps
# the tail (last desc-gen -> DMA -> write) short.
CHUNKS = [256, 512, 1024, 2048, 2048, 1280, 768, 256]
assert sum(CHUNKS) == QTOK
COFFS = [sum(CHUNKS[:i]) for i in range(len(CHUNKS))]
MAXCHUNK = max(CHUNKS)
NBUF = 3
SINGLE_PACKET = False
# ring carveout: per (engine, ctx) partition, DMA_SCRATCH/64 descriptors.
# Worst in-flight per queue = NBUF chunks * (MAXCHUNK/16 + 1) descs = 387.
DMA_SCRATCH = 65536


def _plan_batch(coords_b):
    """Host-side planning from coords only. coords_b: [L,3] int32."""
    x = coords_b[:, 0].astype(np.int64)
    y = coords_b[:, 1].astype(np.int64)
    z = coords_b[:, 2].astype(np.int64)
    keys = ((x * S + y) * S + z).astype(np.int32)
    uniq, first_idx, inv = np.unique(keys, return_index=True, return_inverse=True)
    U = len(uniq)
    src = np.zeros(L, dtype=np.int64)
    src[:U] = first_idx
    # dma_gather token i of a chunk fetches the row for slot base+(i%P)*tpp+i//P
    # (so each SBUF partition holds tpp consecutive slots -> contiguous writes)
    gidx = np.zeros((P, L // 16), dtype=np.int16)
    for q in range(NQ):
        for coff, size in zip(COFFS, CHUNKS):
            base = q * QTOK + coff
            tpp = size // P
            i = np.arange(size)
            slot_local = (i % P) * tpp + i // P
            tokens = src[base + slot_local]
            wrapped = tokens.reshape(size // 16, 16).T.astype(np.int16)
            # 16-partition wrap, replicated for the 8 GPSIMD cores
            gidx[:, base // 16:(base + size) // 16] = np.tile(wrapped, (8, 1))
    dup_mask = np.ones(L, bool)
    dup_mask[first_idx] = False
    dup_points = np.nonzero(dup_mask)[0]
    return dict(U=U, gidx=gidx, dup_points=dup_points,
                dup_rows=first_idx[inv[dup_points]])


def _schedule():
    """(queue, chunk, slot_base, size) in gpsimd/sync issue order."""
    out = []
    for c, (coff, size) in enumerate(zip(COFFS, CHUNKS)):
        for q in range(NQ):
            out.append((q, c, q * QTOK + coff, size))
    return out


def _build_nc():
    from contextlib import ExitStack
    from concourse import bacc, mybir
    from concourse.library_config import mlp

    nc = bacc.Bacc("TRN2", target_bir_lowering=False, debug=False, num_devices=B,
                   dynamic_dma_scratch_size=DMA_SCRATCH, num_swdge_queues=NQ)
    f32, i16 = mybir.dt.float32, mybir.dt.int16
    feats = nc.dram_tensor("feats", [L, C], f32, kind="ExternalInput")
    gidx = nc.dram_tensor("gidx", [P, L // 16], i16, kind="ExternalInput")
    out = nc.dram_tensor("out", [L, C], f32, kind="ExternalOutput")

    sched = _schedule()
    nchunks = len(CHUNKS)

    with (
        nc.Block() as block,
        nc.sbuf_tensor("gidx_sb", [P, L // 16], i16) as gidx_sb,
        nc.sbuf_tensor("gt", [P, NQ, NBUF, MAXCHUNK // P, C], f32) as gt,
        nc.semaphore("io") as io,
        ExitStack() as stack,
    ):
        gs = [[stack.enter_context(nc.semaphore(f"gs{q}_{j}"))  # noqa: ANT232
               for j in range(NBUF)] for q in range(NQ)]
        ws = [[stack.enter_context(nc.semaphore(f"ws{q}_{j}"))  # noqa: ANT232
               for j in range(NBUF)] for q in range(NQ)]

        @block.gpsimd
        def _(gpsimd):
            gpsimd.load_library(mlp)
            gpsimd.wait_ge(io, 16)  # gidx loaded by sync engine
            for q, c, base, size in sched:
                j = c % NBUF
                if c >= NBUF:
                    gpsimd.wait_ge(ws[q][j], 16 * (c // NBUF))
                gpsimd.dma_gather(
                    gt[:, q, j, :size // P], feats[:],
                    gidx_sb[:, base // 16:(base + size) // 16],
                    size, size, C, single_packet=SINGLE_PACKET,
                    queue_num=q,
                ).then_inc(gs[q][j], 16)

        @block.sync
        def _(sync):
            sync.dma_start(gidx_sb[:], gidx[:]).then_inc(io, 16)
            for q, c, base, size in sched:
                j = c % NBUF
                sync.wait_ge(gs[q][j], 16 * (c // NBUF + 1))
                sync.dma_start(
                    out[base:base + size, :].rearrange("(p t) c -> p (t c)", p=P),
                    gt[:, q, j, :size // P],
                ).then_inc(ws[q][j], 16)
            for q in range(NQ):
                for j in range(NBUF):
                    n = sum(1 for c in range(nchunks) if c % NBUF == j)
                    sync.wait_ge(ws[q][j], 16 * n)

    nc.compile()
    return nc


_NC_CACHE = {}
_LAST_RESULTS = {}


def kernel(coords, features):
    from concourse.bass_utils import run_bass_kernel_spmd

    coords = np.asarray(coords)
    features = np.ascontiguousarray(np.asarray(features, dtype=np.float32))
    plans = [_plan_batch(coords[b]) for b in range(B)]
    if 'nc' not in _NC_CACHE:
        _NC_CACHE['nc'] = _build_nc()
    nc = _NC_CACHE['nc']

    in_maps = []
    for b in range(B):
        feats_b = features[b]
        dp = plans[b]['dup_points']
        if len(dp):
            # fold duplicate-point features into their slot's first-occurrence
            # row so the gather alone produces the segment-sum
            orig = feats_b
            feats_b = feats_b.copy()
            np.add.at(feats_b, plans[b]['dup_rows'], orig[dp])
        in_maps.append({"feats": feats_b, "gidx": plans[b]['gidx']})

    import os
    trace = bool(os.environ.get("KERNEL_TRACE_DIR"))
    kw = {}
    if trace:
        try:
            import sys, types
            import antenv
            from trn_agent_boot.trn_boot import _ntff_profile_via_ctypes
            _h = _ntff_profile_via_ctypes('/opt/axon/libaxon_pjrt.so')
            mod = types.ModuleType('antenv.axon_hooks')
            mod.get_axon_ntff_profile_hook = (
                lambda: (lambda outdir, ids: _h(outdir, None)))
            mod.set_axon_ntff_profile_hook = lambda h: None
            sys.modules['antenv.axon_hooks'] = mod
            antenv.axon_hooks = mod
            import concourse.bass_utils as _bu
            _bu.upload_artifacts = lambda tmpdir: tmpdir
            import shutil
            shutil.rmtree(os.environ["KERNEL_TRACE_DIR"], ignore_errors=True)
            os.makedirs(os.environ["KERNEL_TRACE_DIR"], exist_ok=True)
            kw = dict(trace=True, trace_cores=[0],
                      tmpdir=os.environ["KERNEL_TRACE_DIR"])
        except Exception:
            kw = {}

    res = None
    for attempt in range(3):
        try:
            res = run_bass_kernel_spmd(nc, in_maps, core_ids=list(range(B)), **kw)
            break
        except Exception:
            # transient NRT exec-unit errors recover on the next attempt
            if attempt == 2:
                raise
    _LAST_RESULTS['exec_time_ns'] = res.exec_time_ns

    full = np.zeros((B * L, C), np.float32)
    off = 0
    for b in range(B):
        U = plans[b]['U']
        full[off:off + U] = res.results[b]["out"][:U]
        off += U
    return full


# revision 7
# speedup vs baseline: 1.0968x; 1.0968x over previous
"""Trainium2 kernel for nn_BLInputLayer (SparseConvNet mode-3 input layer).

reference semantics: linearize each point's (batch, x, y, z) into a key,
jnp.unique the keys (sorted, size=n, fill -1), segment-sum features by the
inverse index.  Output row u is the feature-sum of the points at the u-th
smallest unique site key; rows past the number of unique sites are zero.

Distribution: data-parallel over the batch dim (8 batches -> 8 NeuronCores).
Keys are batch-major, so the globally sorted unique sites are the per-batch
sorted unique sites concatenated; the host packs the per-core results at the
right row offsets.

Device kernel (per core, raw Bass): the 32768 output slots are produced by
tiled SWDGE `dma_gather`s (one 512B feature row per slot, fetched from the
slot's first occurrence point) pipelined with contiguous HWDGE writes.  The
gathers are spread over 4 SWDGE queues so all 8 GPSIMD cores (one tx/rx pair
per queue) generate DMA descriptors concurrently -- descriptor generation on
a single pair is the sequential bottleneck (~8 ns/row).  The handful of
duplicate points per batch are pre-summed on the host into the uploaded copy
of that batch's features (first-occurrence rows), so the gather alone yields
the exact segment-sum and no device-side read-modify-write pass is needed.
Host work stays O(L) integer planning on coords plus O(dups * C) feature
touches; all bulk feature traffic (16.7 MB in + 16.7 MB out per core) is
on-device.
"""

import numpy as np

B, L, DIM, C = 8, 32768, 3, 128
S = 512
P = 128
NQ = 4                      # SWDGE queues == GPSIMD core pairs used
QTOK = L // NQ              # tokens (output slots) per queue
# Per-queue chunk schedule. A chunk's gather DMA only fires once its
# descriptor generation ends, so tiny front chunks get the DMA engines going
# early; big middle chunks amortize per-instruction overhead; the taper keeps
# the tail (last desc-gen -> DMA -> write) short.
CHUNKS = [512, 1024, 2048, 2048, 2048, 512]
assert sum(CHUNKS) == QTOK
COFFS = [sum(CHUNKS[:i]) for i in range(len(CHUNKS))]
MAXCHUNK = max(CHUNKS)
NBUF = 3
SINGLE_PACKET = False
# ring carveout: per (engine, ctx) partition, DMA_SCRATCH/64 descriptors.
# Worst in-flight per queue = NBUF chunks * (MAXCHUNK/16 + 1) descs = 387.
DMA_SCRATCH = 65536


def _plan_batch(coords_b):
    """Host-side planning from coords only. coords_b: [L,3] int32."""
    x = coords_b[:, 0].astype(np.int64)
    y = coords_b[:, 1].astype(np.int64)
    z = coords_b[:, 2].astype(np.int64)
    keys = ((x * S + y) * S + z).astype(np.int32)
    uniq, first_idx, inv = np.unique(keys, return_index=True, return_inverse=True)
    U = len(uniq)
    src = np.zeros(L, dtype=np.int64)
    src[:U] = first_idx
    # dma_gather token i of a chunk fetches the row for slot base+(i%P)*tpp+i//P
    # (so each SBUF partition holds tpp consecutive slots -> contiguous writes)
    gidx = np.zeros((P, L // 16), dtype=np.int16)
    for q in range(NQ):
        for coff, size in zip(COFFS, CHUNKS):
            base = q * QTOK + coff
            tpp = size // P
            i = np.arange(size)
            slot_local = (i % P) * tpp + i // P
            tokens = src[base + slot_local]
            wrapped = tokens.reshape(size // 16, 16).T.astype(np.int16)
            # 16-partition wrap, replicated for the 8 GPSIMD cores
            gidx[:, base // 16:(base + size) // 16] = np.tile(wrapped, (8, 1))
    dup_mask = np.ones(L, bool)
    dup_mask[first_idx] = False
    dup_points = np.nonzero(dup_mask)[0]
    return dict(U=U, gidx=gidx, dup_points=dup_points,
                dup_rows=first_idx[inv[dup_points]])


def _schedule():
    """(queue, chunk, slot_base, size) in gpsimd/sync issue order."""
    out = []
    for c, (coff, size) in enumerate(zip(COFFS, CHUNKS)):
        for q in range(NQ):
            out.append((q, c, q * QTOK + coff, size))
    return out


def _build_nc():
    from contextlib import ExitStack
    from concourse import bacc, mybir
    from concourse.library_config import mlp

    nc = bacc.Bacc("TRN2", target_bir_lowering=False, debug=False, num_devices=B,
                   dynamic_dma_scratch_size=DMA_SCRATCH, num_swdge_queues=NQ)
    f32, i16 = mybir.dt.float32, mybir.dt.int16
    feats = nc.dram_tensor("feats", [L, C], f32, kind="ExternalInput")
    gidx = nc.dram_tensor("gidx", [P, L // 16], i16, kind="ExternalInput")
    out = nc.dram_tensor("out", [L, C], f32, kind="ExternalOutput")

    sched = _schedule()
    nchunks = len(CHUNKS)

    with (
        nc.Block() as block,
        nc.sbuf_tensor("gidx_sb", [P, L // 16], i16) as gidx_sb,
        nc.sbuf_tensor("gt", [P, NQ, NBUF, MAXCHUNK // P, C], f32) as gt,
        nc.sbuf_tensor("warm", [P, 1, C], f32) as warm,
        nc.sbuf_tensor("warm_idx", [P, 1], i16) as warm_idx,
        nc.semaphore("io") as io,
        nc.semaphore("warmsem") as warmsem,
        ExitStack() as stack,
    ):
        gs = [[stack.enter_context(nc.semaphore(f"gs{q}_{j}"))  # noqa: ANT232
               for j in range(NBUF)] for q in range(NQ)]
        ws = [[stack.enter_context(nc.semaphore(f"ws{q}_{j}"))  # noqa: ANT232
               for j in range(NBUF)] for q in range(NQ)]

        @block.gpsimd
        def _(gpsimd):
            gpsimd.load_library(mlp)
            # Warmup: a 16-token gather per queue, issued before the index
            # table arrives, pulls the one-time SWDGE/DGE queue init (~10us of
            # sequencer MMIO) off the critical path. warm_idx is uninitialized
            # SBUF: any non-negative int16 is a valid feats row, negatives are
            # trimmed/skipped, and the fetched rows land in a scratch tile.
            for q in range(NQ):
                gpsimd.dma_gather(
                    warm[:, :1], feats[:], warm_idx[:, :1], 16, 16, C,
                    single_packet=SINGLE_PACKET, queue_num=q,
                ).then_inc(warmsem, 16)
            gpsimd.wait_ge(io, 16)  # gidx loaded by sync engine
            for q, c, base, size in sched:
                j = c % NBUF
                if c >= NBUF:
                    gpsimd.wait_ge(ws[q][j], 16 * (c // NBUF))
                gpsimd.dma_gather(
                    gt[:, q, j, :size // P], feats[:],
                    gidx_sb[:, base // 16:(base + size) // 16],
                    size, size, C, single_packet=SINGLE_PACKET,
                    queue_num=q,
                ).then_inc(gs[q][j], 16)

        @block.sync
        def _(sync):
            sync.dma_start(gidx_sb[:], gidx[:]).then_inc(io, 16)
            for q, c, base, size in sched:
                j = c % NBUF
                sync.wait_ge(gs[q][j], 16 * (c // NBUF + 1))
                sync.dma_start(
                    out[base:base + size, :].rearrange("(p t) c -> p (t c)", p=P),
                    gt[:, q, j, :size // P],
                ).then_inc(ws[q][j], 16)
            for q in range(NQ):
                for j in range(NBUF):
                    n = sum(1 for c in range(nchunks) if c % NBUF == j)
                    sync.wait_ge(ws[q][j], 16 * n)

    nc.compile()
    return nc


_NC_CACHE = {}
_LAST_RESULTS = {}


def kernel(coords, features):
    from concourse.bass_utils import run_bass_kernel_spmd

    coords = np.asarray(coords)
    features = np.ascontiguousarray(np.asarray(features, dtype=np.float32))
    plans = [_plan_batch(coords[b]) for b in range(B)]
    if 'nc' not in _NC_CACHE:
        _NC_CACHE['nc'] = _build_nc()
    nc = _NC_CACHE['nc']

    in_maps = []
    for b in range(B):
        feats_b = features[b]
        dp = plans[b]['dup_points']
        if len(dp):
            # fold duplicate-point features into their slot's first-occurrence
            # row so the gather alone produces the segment-sum
            orig = feats_b
            feats_b = feats_b.copy()
            np.add.at(feats_b, plans[b]['dup_rows'], orig[dp])
        in_maps.append({"feats": feats_b, "gidx": plans[b]['gidx']})

    import os
    trace = bool(os.environ.get("KERNEL_TRACE_DIR"))
    kw = {}
    if trace:
        try:
            import sys, types
            import antenv
            from trn_agent_boot.trn_boot import _ntff_profile_via_ctypes
            _h = _ntff_profile_via_ctypes('/opt/axon/libaxon_pjrt.so')
            mod = types.ModuleType('antenv.axon_hooks')
            mod.get_axon_ntff_profile_hook = (
                lambda: (lambda outdir, ids: _h(outdir, None)))
            mod.set_axon_ntff_profile_hook = lambda h: None
            sys.modules['antenv.axon_hooks'] = mod
            antenv.axon_hooks = mod
            import concourse.bass_utils as _bu
            _bu.upload_artifacts = lambda tmpdir: tmpdir
            import shutil
            shutil.rmtree(os.environ["KERNEL_TRACE_DIR"], ignore_errors=True)
            os.makedirs(os.environ["KERNEL_TRACE_DIR"], exist_ok=True)
            kw = dict(trace=True, trace_cores=[0],
                      tmpdir=os.environ["KERNEL_TRACE_DIR"])
        except Exception:
            kw = {}

    res = None
    for attempt in range(3):
        try:
            res = run_bass_kernel_spmd(nc, in_maps, core_ids=list(range(B)), **kw)
            break
        except Exception:
            # transient NRT exec-unit errors recover on the next attempt
            if attempt == 2:
                raise
    _LAST_RESULTS['exec_time_ns'] = res.exec_time_ns

    full = np.zeros((B * L, C), np.float32)
    off = 0
    for b in range(B):
        U = plans[b]['U']
        full[off:off + U] = res.results[b]["out"][:U]
        off += U
    return full


# revision 8
# speedup vs baseline: 1.0982x; 1.0012x over previous
"""Trainium2 kernel for nn_BLInputLayer (SparseConvNet mode-3 input layer).

reference semantics: linearize each point's (batch, x, y, z) into a key,
jnp.unique the keys (sorted, size=n, fill -1), segment-sum features by the
inverse index.  Output row u is the feature-sum of the points at the u-th
smallest unique site key; rows past the number of unique sites are zero.

Distribution: data-parallel over the batch dim (8 batches -> 8 NeuronCores).
Keys are batch-major, so the globally sorted unique sites are the per-batch
sorted unique sites concatenated; the host packs the per-core results at the
right row offsets.

Device kernel (per core, raw Bass): the 32768 output slots are produced by
tiled SWDGE `dma_gather`s (one 512B feature row per slot, fetched from the
slot's first occurrence point) pipelined with contiguous HWDGE writes.  The
gathers are spread over 4 SWDGE queues so all 8 GPSIMD cores (one tx/rx pair
per queue) generate DMA descriptors concurrently -- descriptor generation on
a single pair is the sequential bottleneck (~8 ns/row).  The handful of
duplicate points per batch are pre-summed on the host into the uploaded copy
of that batch's features (first-occurrence rows), so the gather alone yields
the exact segment-sum and no device-side read-modify-write pass is needed.
Host work stays O(L) integer planning on coords plus O(dups * C) feature
touches; all bulk feature traffic (16.7 MB in + 16.7 MB out per core) is
on-device.
"""

import numpy as np

B, L, DIM, C = 8, 32768, 3, 128
S = 512
P = 128
NQ = 4                      # SWDGE queues == GPSIMD core pairs used
QTOK = L // NQ              # tokens (output slots) per queue
# Per-queue chunk schedule. A chunk's gather DMA only fires once its
# descriptor generation ends, so tiny front chunks get the DMA engines going
# early; big middle chunks amortize per-instruction overhead; the taper keeps
# the tail (last desc-gen -> DMA -> write) short.
CHUNKS = [512, 1024, 2048, 2048, 2048, 512]
assert sum(CHUNKS) == QTOK
COFFS = [sum(CHUNKS[:i]) for i in range(len(CHUNKS))]
MAXCHUNK = max(CHUNKS)
NBUF = 3
SINGLE_PACKET = False
# ring carveout: per (engine, ctx) partition, DMA_SCRATCH/64 descriptors.
# Worst in-flight per queue = NBUF chunks * (MAXCHUNK/16 + 1) descs = 387.
DMA_SCRATCH = 65536


def _plan_batch(coords_b):
    """Host-side planning from coords only. coords_b: [L,3] int32."""
    x = coords_b[:, 0].astype(np.int64)
    y = coords_b[:, 1].astype(np.int64)
    z = coords_b[:, 2].astype(np.int64)
    keys = ((x * S + y) * S + z).astype(np.int32)
    uniq, first_idx, inv = np.unique(keys, return_index=True, return_inverse=True)
    U = len(uniq)
    src = np.zeros(L, dtype=np.int64)
    src[:U] = first_idx
    # dma_gather token i of a chunk fetches the row for slot base+(i%P)*tpp+i//P
    # (so each SBUF partition holds tpp consecutive slots -> contiguous writes)
    gidx = np.zeros((P, L // 16), dtype=np.int16)
    for q in range(NQ):
        for coff, size in zip(COFFS, CHUNKS):
            base = q * QTOK + coff
            tpp = size // P
            i = np.arange(size)
            slot_local = (i % P) * tpp + i // P
            tokens = src[base + slot_local]
            wrapped = tokens.reshape(size // 16, 16).T.astype(np.int16)
            # 16-partition wrap, replicated for the 8 GPSIMD cores
            gidx[:, base // 16:(base + size) // 16] = np.tile(wrapped, (8, 1))
    dup_mask = np.ones(L, bool)
    dup_mask[first_idx] = False
    dup_points = np.nonzero(dup_mask)[0]
    return dict(U=U, gidx=gidx, dup_points=dup_points,
                dup_rows=first_idx[inv[dup_points]])


def _schedule():
    """(queue, chunk, slot_base, size) in gpsimd/sync issue order."""
    out = []
    for c, (coff, size) in enumerate(zip(COFFS, CHUNKS)):
        for q in range(NQ):
            out.append((q, c, q * QTOK + coff, size))
    return out


def _build_nc():
    from contextlib import ExitStack
    from concourse import bacc, mybir
    from concourse.library_config import mlp

    nc = bacc.Bacc("TRN2", target_bir_lowering=False, debug=False, num_devices=B,
                   dynamic_dma_scratch_size=DMA_SCRATCH, num_swdge_queues=NQ)
    f32, i16 = mybir.dt.float32, mybir.dt.int16
    feats = nc.dram_tensor("feats", [L, C], f32, kind="ExternalInput")
    gidx = nc.dram_tensor("gidx", [P, L // 16], i16, kind="ExternalInput")
    out = nc.dram_tensor("out", [L, C], f32, kind="ExternalOutput")

    sched = _schedule()
    nchunks = len(CHUNKS)

    with (
        nc.Block() as block,
        nc.sbuf_tensor("gidx_sb", [P, L // 16], i16) as gidx_sb,
        nc.sbuf_tensor("gt", [P, NQ, NBUF, MAXCHUNK // P, C], f32) as gt,
        nc.sbuf_tensor("warm", [P, 1, C], f32) as warm,
        nc.sbuf_tensor("warm_idx", [P, 1], i16) as warm_idx,
        nc.semaphore("io") as io,
        nc.semaphore("warmsem") as warmsem,
        ExitStack() as stack,
    ):
        gs = [[stack.enter_context(nc.semaphore(f"gs{q}_{j}"))  # noqa: ANT232
               for j in range(NBUF)] for q in range(NQ)]
        ws = [[stack.enter_context(nc.semaphore(f"ws{q}_{j}"))  # noqa: ANT232
               for j in range(NBUF)] for q in range(NQ)]

        @block.gpsimd
        def _(gpsimd):
            gpsimd.load_library(mlp)
            # One register per distinct num_idxs: each to_reg MOVE costs
            # ~400ns of sequencer time, so don't emit one per gather.
            regs = {n: gpsimd.to_reg(n) for n in sorted({16, *CHUNKS})}
            # Warmup: a 16-token gather per queue, issued before the index
            # table arrives, pulls the one-time SWDGE/DGE queue init off the
            # critical path. warm_idx is uninitialized SBUF: any non-negative
            # int16 is a valid feats row, negatives are trimmed/skipped, and
            # the fetched rows land in a scratch tile.
            for q in range(NQ):
                gpsimd.dma_gather(
                    warm[:, :1], feats[:], warm_idx[:, :1], 16, regs[16], C,
                    single_packet=SINGLE_PACKET, queue_num=q,
                ).then_inc(warmsem, 16)
            gpsimd.wait_ge(io, 16)  # gidx loaded by sync engine
            for q, c, base, size in sched:
                j = c % NBUF
                if c >= NBUF:
                    gpsimd.wait_ge(ws[q][j], 16 * (c // NBUF))
                gpsimd.dma_gather(
                    gt[:, q, j, :size // P], feats[:],
                    gidx_sb[:, base // 16:(base + size) // 16],
                    size, regs[size], C, single_packet=SINGLE_PACKET,
                    queue_num=q,
                ).then_inc(gs[q][j], 16)

        @block.sync
        def _(sync):
            sync.dma_start(gidx_sb[:], gidx[:]).then_inc(io, 16)
            for q, c, base, size in sched:
                j = c % NBUF
                sync.wait_ge(gs[q][j], 16 * (c // NBUF + 1))
                sync.dma_start(
                    out[base:base + size, :].rearrange("(p t) c -> p (t c)", p=P),
                    gt[:, q, j, :size // P],
                ).then_inc(ws[q][j], 16)
            for q in range(NQ):
                for j in range(NBUF):
                    n = sum(1 for c in range(nchunks) if c % NBUF == j)
                    sync.wait_ge(ws[q][j], 16 * n)

    nc.compile()
    return nc


_NC_CACHE = {}
_LAST_RESULTS = {}


def kernel(coords, features):
    from concourse.bass_utils import run_bass_kernel_spmd

    coords = np.asarray(coords)
    features = np.ascontiguousarray(np.asarray(features, dtype=np.float32))
    plans = [_plan_batch(coords[b]) for b in range(B)]
    if 'nc' not in _NC_CACHE:
        _NC_CACHE['nc'] = _build_nc()
    nc = _NC_CACHE['nc']

    in_maps = []
    for b in range(B):
        feats_b = features[b]
        dp = plans[b]['dup_points']
        if len(dp):
            # fold duplicate-point features into their slot's first-occurrence
            # row so the gather alone produces the segment-sum
            orig = feats_b
            feats_b = feats_b.copy()
            np.add.at(feats_b, plans[b]['dup_rows'], orig[dp])
        in_maps.append({"feats": feats_b, "gidx": plans[b]['gidx']})

    import os
    trace = bool(os.environ.get("KERNEL_TRACE_DIR"))
    kw = {}
    if trace:
        try:
            import sys, types
            import antenv
            from trn_agent_boot.trn_boot import _ntff_profile_via_ctypes
            _h = _ntff_profile_via_ctypes('/opt/axon/libaxon_pjrt.so')
            mod = types.ModuleType('antenv.axon_hooks')
            mod.get_axon_ntff_profile_hook = (
                lambda: (lambda outdir, ids: _h(outdir, None)))
            mod.set_axon_ntff_profile_hook = lambda h: None
            sys.modules['antenv.axon_hooks'] = mod
            antenv.axon_hooks = mod
            import concourse.bass_utils as _bu
            _bu.upload_artifacts = lambda tmpdir: tmpdir
            import shutil
            shutil.rmtree(os.environ["KERNEL_TRACE_DIR"], ignore_errors=True)
            os.makedirs(os.environ["KERNEL_TRACE_DIR"], exist_ok=True)
            kw = dict(trace=True, trace_cores=[0],
                      tmpdir=os.environ["KERNEL_TRACE_DIR"])
        except Exception:
            kw = {}

    res = None
    for attempt in range(3):
        try:
            res = run_bass_kernel_spmd(nc, in_maps, core_ids=list(range(B)), **kw)
            break
        except Exception:
            # transient NRT exec-unit errors recover on the next attempt
            if attempt == 2:
                raise
    _LAST_RESULTS['exec_time_ns'] = res.exec_time_ns

    full = np.zeros((B * L, C), np.float32)
    off = 0
    for b in range(B):
        U = plans[b]['U']
        full[off:off + U] = res.results[b]["out"][:U]
        off += U
    return full
